# revision 1
# baseline (speedup 1.0000x reference)
"""LoRA attention processor on 8 NeuronCores (Trainium2, Bass/Tile).

Reference computation (B=2, S=4096, D=1280, H=8 heads, dh=160, rank-4 LoRA
on K/V):
    q = x @ Wq; k = x @ Wk; v = x @ Wv
    k += (k @ Ak) @ Bk; v += (v @ Av) @ Bv        (LoRA, rank 4)
    attn = softmax(q k^T / sqrt(dh)) v   per head
    out = attn @ Wout + b_out

Sharding: core c handles batch b = c//4 and head pair p = c%4 (columns
320p:320p+320 of the QKV projections, rows of Wout). The LoRA update is
folded into the weights on the host: k + (k@Ak)@Bk == x @ (Wk + Wk@Ak@Bk),
so each core only needs its 320-column slice of the effective weights.
Each core returns a partial output (its heads' contribution to attn@Wout);
the host sums the 4 partials per batch and adds the bias.

On-core layout: scores are computed transposed ([k-pos partitions, q-pos
free]) so softmax's exp runs on ACT over PSUM directly and the PV matmul
needs no transposes: outT[d, q] = sum_j V[j, d] * expT[j, q]. The softmax
denominator rides along as a ones-column appended to V (row 160 of the PV
output), and normalization is applied to outT (160 x 4096 per head)
instead of to the 4096 x 4096 probability matrix. No row-max subtraction:
scores are ~N(0,1) here (|s| < ~7), exp cannot overflow fp32.

All big matmuls run in float32r (TF32-style reduced-precision fp32, full
PE rate at free-dim >= 256 vs 4x slower for exact fp32).
"""

import numpy as np
import ml_dtypes
from contextlib import ExitStack

import concourse.bass as bass
import concourse.tile as tile
from concourse import bacc, mybir
from concourse.bass_utils import run_bass_kernel_spmd

B, S, D = 2, 4096, 1280
H, DH = 8, 160
HP = 320           # head-pair columns per core (2 heads)
N_CORES = 8
SC = 512           # free-dim chunk (q columns / s columns)
NSC = S // SC      # 8
CK = 128           # contraction chunk
NCK = D // CK      # 10
F32 = mybir.dt.float32
F32R = mybir.dt.float32r
BF16 = mybir.dt.bfloat16

_CACHE = {}


def build():
    nc = bacc.Bacc("TRN2", target_bir_lowering=False, debug=False,
                   num_devices=N_CORES)
    # inputs (float32r decl == fp32 bits; PE reads reduced precision)
    xT = nc.dram_tensor("xT", [D, S], F32R, kind="ExternalInput").ap()
    wq = nc.dram_tensor("wq", [D, HP], F32R, kind="ExternalInput").ap()
    wk = nc.dram_tensor("wk", [D, HP], F32R, kind="ExternalInput").ap()
    wv = nc.dram_tensor("wv", [D, HP], F32R, kind="ExternalInput").ap()
    wo = nc.dram_tensor("wo", [HP, D], F32R, kind="ExternalInput").ap()
    ones2 = nc.dram_tensor("ones2", [1, 128], F32, kind="ExternalInput").ap()
    # [...,0]=1 feeds the denominator row of the PV matmul; [...,1]=0 pads
    # the V free dim to an even size (fp32r layout rule)
    onesv = nc.dram_tensor("onesv", [128, 32, 2], F32R, kind="ExternalInput").ap()
    out = nc.dram_tensor("out", [S, D], F32, kind="ExternalOutput").ap()
    # scratch
    qT_d = nc.dram_tensor("qT_d", [HP, S], F32R).ap()
    oT_d = nc.dram_tensor("oT_d", [HP, S], F32R).ap()

    # per-head row chunks of the 320-wide slice: (offset, size)
    hchunks = [[(0, 128), (128, 32)], [(160, 128), (288, 32)]]

    with tile.TileContext(nc) as tc, ExitStack() as top:
        kt_pool = top.enter_context(tc.tile_pool(name="kt", bufs=1))
        v_pool = top.enter_context(tc.tile_pool(name="vp", bufs=1))
        KT = [kt_pool.tile([sz, S], F32R, name=f"KT{i}", tag=f"KT{i}")
              for i, (_, sz) in enumerate(hchunks[0] + hchunks[1])]
        V = [v_pool.tile([128, 32, 162], F32R, name=f"V{h}", tag=f"V{h}")
             for h in range(2)]

        # ---- phase 1: projections QT/KT (transposed) + V (natural) ----
        with ExitStack() as ph1:
            xp = ph1.enter_context(tc.tile_pool(name="xp", bufs=2))
            wp = ph1.enter_context(tc.tile_pool(name="wp", bufs=1))
            pp = ph1.enter_context(tc.tile_pool(name="pp", bufs=4, space="PSUM"))
            sp = ph1.enter_context(tc.tile_pool(name="sp", bufs=3))

            warm = sp.tile([1, 2], F32, tag="warm")
            nc.vector.memset(warm[:], 0.0)
            warm2 = sp.tile([1, 2], F32, tag="warm2")
            nc.scalar.activation(warm2[:], warm[:],
                                 mybir.ActivationFunctionType.Exp)
            wts = {}
            for nm, src in (("wq", wq), ("wk", wk), ("wv", wv)):
                for c in range(NCK):
                    t = wp.tile([CK, HP], F32R, name=f"{nm}_{c}", tag=f"{nm}_{c}")
                    nc.sync.dma_start(t[:], src[c * CK:(c + 1) * CK, :])
                    wts[(nm, c)] = t
            for h in range(2):
                nc.sync.dma_start(V[h][:, :, 160:162], onesv[:])

            for sc in range(NSC):
                xts = []
                for c in range(NCK):
                    xt = xp.tile([CK, SC], F32R, tag=f"xt{c}")
                    nc.sync.dma_start(xt[:], xT[c * CK:(c + 1) * CK,
                                                 sc * SC:(sc + 1) * SC])
                    xts.append(xt)
                # QT / KT chunks: psum[m, q] = sum_c w[c, m].T @ xT[c, q]
                for nm, dst in (("wq", None), ("wk", KT)):
                    for i, (off, msz) in enumerate(hchunks[0] + hchunks[1]):
                        ps = pp.tile([msz, SC], F32, tag="ps")
                        for c in range(NCK):
                            nc.tensor.matmul(
                                ps[:], wts[(nm, c)][:, off:off + msz], xts[c][:],
                                start=(c == 0), stop=(c == NCK - 1))
                        st = sp.tile([msz, SC], F32R, tag=f"st{msz}")
                        nc.vector.tensor_copy(st[:], ps[:])
                        if dst is None:
                            nc.sync.dma_start(
                                qT_d[off:off + msz, sc * SC:(sc + 1) * SC], st[:])
                        else:
                            nc.vector.tensor_copy(
                                dst[i][:, sc * SC:(sc + 1) * SC], ps[:])
                # V natural: psum[s, dv] = xT[c, s].T @ wv[c, :]
                for st4 in range(4):
                    s0 = sc * 4 + st4
                    ps = pp.tile([128, HP], F32, tag="psv")
                    for c in range(NCK):
                        nc.tensor.matmul(
                            ps[:], xts[c][:, st4 * 128:(st4 + 1) * 128],
                            wts[("wv", c)][:], start=(c == 0), stop=(c == NCK - 1))
                    for h in range(2):
                        nc.vector.tensor_copy(V[h][:, s0, 0:160],
                                              ps[:, h * 160:(h + 1) * 160])

        # ---- phase 2: attention per head ----
        with ExitStack() as ph2:
            qp = ph2.enter_context(tc.tile_pool(name="qp", bufs=2))
            scp = ph2.enter_context(tc.tile_pool(name="scp", bufs=3, space="PSUM"))
            ovp = ph2.enter_context(tc.tile_pool(name="ovp", bufs=2, space="PSUM"))
            rbp = ph2.enter_context(tc.tile_pool(name="rbp", bufs=1, space="PSUM"))
            ep = ph2.enter_context(tc.tile_pool(name="ep", bufs=3))
            np_ = ph2.enter_context(tc.tile_pool(name="np", bufs=2))
            o2 = ph2.enter_context(tc.tile_pool(name="o2", bufs=1))
            ones2_t = o2.tile([1, 128], F32)
            nc.sync.dma_start(ones2_t[:], ones2[:])

            qts = {}
            for h in range(2):
                (offA, _), (offB, _) = hchunks[h]
                for qc in range(NSC):
                    qs = slice(qc * SC, (qc + 1) * SC)
                    qA = qp.tile([128, SC], F32R, tag=f"qA{h}_{qc}", bufs=1)
                    qB = qp.tile([32, SC], F32R, tag=f"qB{h}_{qc}", bufs=1)
                    nc.sync.dma_start(qA[:], qT_d[offA:offA + 128, qs])
                    nc.sync.dma_start(qB[:], qT_d[offB:offB + 32, qs])
                    qts[(h, qc)] = (qA, qB)

            for h in range(2):
                (offA, _), (offB, _) = hchunks[h]
                ktA, ktB = KT[2 * h], KT[2 * h + 1]
                for qc in range(NSC):
                    qs = slice(qc * SC, (qc + 1) * SC)
                    qA, qB = qts[(h, qc)]
                    oA = ovp.tile([128, SC], F32, tag="oA")
                    oB = ovp.tile([34, SC], F32, tag="oB")
                    for j in range(32):
                        js = slice(j * 128, (j + 1) * 128)
                        sc_ps = scp.tile([128, SC], F32, tag="sc")
                        nc.tensor.matmul(sc_ps[:], ktA[:, js], qA[:],
                                         start=True, stop=False)
                        nc.tensor.matmul(sc_ps[:], ktB[:, js], qB[:],
                                         start=False, stop=True)
                        ex = ep.tile([128, SC], F32R, tag="ex")
                        nc.scalar.activation(ex[:], sc_ps[:],
                                             mybir.ActivationFunctionType.Exp)
                        nc.tensor.matmul(oA[:], V[h][:, j, 0:128], ex[:],
                                         start=(j == 0), stop=(j == 31))
                        nc.tensor.matmul(oB[:], V[h][:, j, 128:162], ex[:],
                                         start=(j == 0), stop=(j == 31))
                    # normalize by denominator (row 32 of oB) and store
                    rec = np_.tile([1, SC], F32, tag="rec")
                    nc.vector.reciprocal(rec[:], oB[32:33, :])
                    rb = rbp.tile([128, SC], F32, tag="rb")
                    nc.tensor.matmul(rb[:], ones2_t[:], rec[:],
                                     start=True, stop=True)
                    rbs = np_.tile([128, SC], F32, tag="rbs")
                    nc.vector.tensor_copy(rbs[:], rb[:])
                    onA = np_.tile([128, SC], F32R, tag="onA")
                    onB = np_.tile([32, SC], F32R, tag="onB")
                    nc.vector.tensor_mul(onA[:], oA[:], rbs[:])
                    nc.vector.tensor_mul(onB[:], oB[0:32, :], rbs[0:32, :])
                    nc.sync.dma_start(oT_d[offA:offA + 128, qs], onA[:])
                    nc.sync.dma_start(oT_d[offB:offB + 32, qs], onB[:])

        # ---- phase 3: output projection (partial over this core's cols) ----
        with ExitStack() as ph3:
            op = ph3.enter_context(tc.tile_pool(name="op", bufs=1))
            wop = ph3.enter_context(tc.tile_pool(name="wop", bufs=1))
            fp = ph3.enter_context(tc.tile_pool(name="fp", bufs=4, space="PSUM"))
            fs = ph3.enter_context(tc.tile_pool(name="fs", bufs=3))
            chunks = hchunks[0] + hchunks[1]
            woc = []
            for i, (off, msz) in enumerate(chunks):
                w = wop.tile([msz, D], F32R, name=f"wo{i}", tag=f"wo{i}")
                nc.sync.dma_start(w[:], wo[off:off + msz, :])
                woc.append(w)
            for qc in range(NSC):
                qs = slice(qc * SC, (qc + 1) * SC)
                oTc = []
                for i, (off, msz) in enumerate(chunks):
                    t = op.tile([msz, SC], F32R, tag=f"oT{i}", bufs=2)
                    nc.sync.dma_start(t[:], oT_d[off:off + msz, qs])
                    oTc.append(t)
                for st4 in range(4):
                    ss = slice(st4 * 128, (st4 + 1) * 128)
                    row = qc * SC + st4 * 128
                    ot = fs.tile([128, D], F32, tag="ot")
                    for oc, osz in ((0, 512), (512, 512), (1024, 256)):
                        ps = fp.tile([128, osz], F32, tag=f"fo{osz}")
                        for i in range(4):
                            nc.tensor.matmul(ps[:], oTc[i][:, ss],
                                             woc[i][:, oc:oc + osz],
                                             start=(i == 0), stop=(i == 3))
                        nc.vector.tensor_copy(ot[:, oc:oc + osz], ps[:])
                    nc.sync.dma_start(out[row:row + 128, :], ot[:])

    nc.compile()
    return nc


def kernel(hidden_states, w_q, w_k, w_v, lora_k_a, lora_k_b,
           lora_v_a, lora_v_b, w_out, b_out):
    f64 = np.float64
    wk_eff = (w_k.astype(f64)
              + w_k.astype(f64) @ lora_k_a.astype(f64) @ lora_k_b.astype(f64)
              ).astype(np.float32)
    wv_eff = (w_v.astype(f64)
              + w_v.astype(f64) @ lora_v_a.astype(f64) @ lora_v_b.astype(f64)
              ).astype(np.float32)
    wq_s = (w_q.astype(f64) / np.sqrt(DH)).astype(np.float32)

    ones2 = np.ones((1, 128), np.float32)
    onesv = np.zeros((128, 32, 2), np.float32)
    onesv[:, :, 0] = 1.0
    xT = [np.ascontiguousarray(np.asarray(hidden_states)[b].T) for b in range(B)]

    in_maps = []
    for c in range(N_CORES):
        b, p = c // 4, c % 4
        cols = slice(p * HP, (p + 1) * HP)
        in_maps.append({
            "xT": xT[b],
            "wq": np.ascontiguousarray(wq_s[:, cols]),
            "wk": np.ascontiguousarray(wk_eff[:, cols]),
            "wv": np.ascontiguousarray(wv_eff[:, cols]),
            "wo": np.ascontiguousarray(w_out[cols, :]),
            "ones2": ones2,
            "onesv": onesv,
        })

    global _last_in_maps
    _last_in_maps = in_maps
    if "nc" not in _CACHE:
        _CACHE["nc"] = build()
    res = run_bass_kernel_spmd(_CACHE["nc"], in_maps, list(range(N_CORES)))

    out = np.zeros((B, S, D), np.float32)
    for c in range(N_CORES):
        out[c // 4] += res.results[c]["out"]
    out += np.asarray(b_out, np.float32)
    return out



# revision 16
# speedup vs baseline: 2.1757x; 2.1757x over previous
"""LoRA attention processor on 8 NeuronCores (Trainium2, Bass/Tile).

Reference computation (B=2, S=4096, D=1280, H=8 heads, dh=160, rank-4 LoRA
on K/V):
    q = x @ Wq; k = x @ Wk; v = x @ Wv
    k += (k @ Ak) @ Bk; v += (v @ Av) @ Bv        (LoRA, rank 4)
    attn = softmax(q k^T / sqrt(dh)) v   per head
    out = attn @ Wout + b_out

Sharding: core c handles batch b = c//4 and head pair p = c%4. The LoRA
update is folded into the weights on the host. Each core returns a partial
output (its heads' contribution to attn@Wout); the host sums the 4 partials
per batch and adds the bias.

All matmuls run in fp16 (full PE rate, 3 more mantissa bits than bf16).
exp() is computed with a constant -5.5 shift so e^smax (~15.5) fits fp16;
the shift cancels in the softmax normalization. Weight columns per head
pair are reordered [h0 dims 0:128 | h1 dims 0:128 | h0 128:160 | h1
128:160] so the dh=160 tails pack: score tail matmuls are K=32 row-tiles
at tile_position (0,0)/(32,0) (concurrent), PV tail matmuls are M=33
col-tiles at (0,0)/(0,64) (concurrent), with the softmax-denominator ones
column riding in the M=33 aug weights. The [k-pos, q-pos] transposed
score layout lets exp run on ACT straight out of PSUM ([128,1024] wide
read spanning both heads' banks) and the PV matmuls need no transposes.
"""

import numpy as np
from contextlib import ExitStack

import concourse.bass as bass
import concourse.tile as tile
from concourse import bacc, mybir
from concourse.bass_utils import run_bass_kernel_spmd

B, S, D = 2, 4096, 1280
H, DH = 8, 160
HP = 320           # head-pair columns per core (2 heads)
N_CORES = 8
SC = 512           # free-dim chunk (q columns)
NSC = S // SC      # 8
CK = 128           # contraction chunk
NCK = D // CK      # 10
NJ = S // 128      # 32 key blocks
F32 = mybir.dt.float32
F32R = mybir.dt.float32r
F16 = mybir.dt.float16
SHIFT = 5.5        # exp(s - SHIFT): keeps e^s within fp16 range

_CACHE = {}


def build():
    nc = bacc.Bacc("TRN2", target_bir_lowering=False, debug=False,
                   num_devices=N_CORES)
    xT = nc.dram_tensor("xT", [D, S], F16, kind="ExternalInput").ap()
    wq = nc.dram_tensor("wq", [D, HP], F16, kind="ExternalInput").ap()
    wk = nc.dram_tensor("wk", [D, HP], F16, kind="ExternalInput").ap()
    wv = nc.dram_tensor("wv", [D, HP], F16, kind="ExternalInput").ap()
    # rows: [h0 dims 0:128 | h1 dims 0:128 | h0B@0:32 + h1B@64:96, zero-pad]
    wo = nc.dram_tensor("wo", [384, D], F16, kind="ExternalInput").ap()
    out = nc.dram_tensor("out", [S, D], F32, kind="ExternalOutput").ap()

    ExpF = mybir.ActivationFunctionType.Exp

    with tile.TileContext(nc) as tc, ExitStack() as top:
        per = top.enter_context(tc.tile_pool(name="per", bufs=1))
        # persistent SBUF tiles (fp16)
        QA0 = per.tile([128, S], F16, name="QA0", tag="QA0")
        QA1 = per.tile([128, S], F16, name="QA1", tag="QA1")
        QB = per.tile([64, S], F16, name="QB", tag="QB")
        KA0 = per.tile([128, S], F16, name="KA0", tag="KA0")
        KA1 = per.tile([128, S], F16, name="KA1", tag="KA1")
        KB = per.tile([64, S], F16, name="KB", tag="KB")
        VA0 = per.tile([128, NJ, 128], F16, name="VA0", tag="VA0")
        VA1 = per.tile([128, NJ, 128], F16, name="VA1", tag="VA1")
        # V tails + denominator ones: [:, j, 0:32]=h0 dims, 32=ones, 33=zero,
        # 34:66=h1 dims, 66=ones, 67=zero  (aug lhsT slices 0:34 and 34:68,
        # even M=34 to satisfy the 4-byte stationary-operand layout rule)
        VB = per.tile([128, NJ, 68], F16, name="VB", tag="VB")
        # normalized attention output, transposed [dh, q]
        oTA0 = per.tile([128, S], F16, name="oTA0", tag="oTA0")
        oTA1 = per.tile([128, S], F16, name="oTA1", tag="oTA1")
        # tails: h0 dims at partitions 0:32, h1 at 64:96; rest stays zero
        oTB = per.tile([128, S], F16, name="oTB", tag="oTB")
        ones_rb = per.tile([1, 128], F16, name="ones_rb", tag="ones_rb")
        # output projection weights
        WO = [per.tile([128, D], F16, name=f"WO{i}", tag=f"WO{i}")
              for i in range(3)]

        bias_t = per.tile([128, 1], F32, name="bias_t", tag="bias_t")
        nc.vector.memset(bias_t[:], -SHIFT)
        nc.vector.memset(ones_rb[:], 1.0)
        nc.vector.memset(VB[:, :, 32:33], 1.0)
        nc.vector.memset(VB[:, :, 33:34], 0.0)
        nc.vector.memset(VB[:, :, 66:67], 1.0)
        nc.vector.memset(VB[:, :, 67:68], 0.0)
        nc.gpsimd.memset(oTB[:], 0.0)
        for i in range(3):
            nc.sync.dma_start(WO[i][:], wo[i * 128:(i + 1) * 128, :])

        # warm the ACT exp table early so it's off phase 2's critical path
        warm = per.tile([1, 2], F32, name="warm", tag="warm")
        nc.vector.memset(warm[:], 0.0)
        warm2 = per.tile([1, 2], F32, name="warm2", tag="warm2")
        nc.scalar.activation(warm2[:], warm[:], ExpF)

        # ---- phase 1: projections QT/KT (transposed [dh,q]) + V natural ----
        with ExitStack() as ph1:
            xp = ph1.enter_context(tc.tile_pool(name="xp", bufs=2))
            wp = ph1.enter_context(tc.tile_pool(name="wp", bufs=1))
            pq = ph1.enter_context(tc.tile_pool(name="pq", bufs=2, space="PSUM"))
            pv = ph1.enter_context(tc.tile_pool(name="pv", bufs=2, space="PSUM"))

            wts = {}
            for nm, src in (("wq", wq), ("wk", wk), ("wv", wv)):
                for c in range(NCK):
                    t = wp.tile([CK, HP], F16, name=f"{nm}_{c}", tag=f"{nm}_{c}")
                    nc.sync.dma_start(t[:], src[c * CK:(c + 1) * CK, :])
                    wts[(nm, c)] = t

            qk_dsts = {"wq": (QA0, QA1, QB), "wk": (KA0, KA1, KB)}
            for sc in range(NSC):
                ss = slice(sc * SC, (sc + 1) * SC)
                xts = []
                for c in range(NCK):
                    xt = xp.tile([CK, SC], F16, tag=f"xt{c}")
                    nc.sync.dma_start(xt[:], xT[c * CK:(c + 1) * CK, ss])
                    xts.append(xt)
                for nm in ("wq", "wk"):
                    dA0, dA1, dB = qk_dsts[nm]
                    psA0 = pq.tile([128, SC], F32, tag="pA0")
                    psA1 = pq.tile([128, SC], F32, tag="pA1")
                    psB = pq.tile([64, SC], F32, tag="pB")
                    for c in range(NCK):
                        st, sp_ = (c == 0), (c == NCK - 1)
                        nc.tensor.matmul(psA0[:], wts[(nm, c)][:, 0:128],
                                         xts[c][:], start=st, stop=sp_)
                    for c in range(NCK):
                        st, sp_ = (c == 0), (c == NCK - 1)
                        nc.tensor.matmul(psA1[:], wts[(nm, c)][:, 128:256],
                                         xts[c][:], start=st, stop=sp_)
                    for c in range(NCK):
                        st, sp_ = (c == 0), (c == NCK - 1)
                        nc.tensor.matmul(psB[:], wts[(nm, c)][:, 256:320],
                                         xts[c][:], start=st, stop=sp_)
                    nc.any.tensor_copy(dA0[:, ss], psA0[:])
                    nc.any.tensor_copy(dA1[:, ss], psA1[:])
                    nc.any.tensor_copy(dB[:, ss], psB[:])
                # V natural: psum[s, d] = x[c, s].T @ wv[c, :]
                for st4 in range(4):
                    s0 = sc * 4 + st4
                    psV = pv.tile([128, HP], F32, tag="pV")
                    for c in range(NCK):
                        nc.tensor.matmul(
                            psV[:], xts[c][:, st4 * 128:(st4 + 1) * 128],
                            wts[("wv", c)][:], start=(c == 0), stop=(c == NCK - 1))
                    nc.vector.tensor_copy(VA0[:, s0, :], psV[:, 0:128])
                    nc.vector.tensor_copy(VA1[:, s0, :], psV[:, 128:256])
                    nc.vector.tensor_copy(VB[:, s0, 0:32], psV[:, 256:288])
                    nc.vector.tensor_copy(VB[:, s0, 34:66], psV[:, 288:320])

        # ---- phase 2: attention, both heads together per q-chunk ----
        with ExitStack() as ph2:
            scp = ph2.enter_context(tc.tile_pool(name="scp", bufs=2, space="PSUM"))
            ovp = ph2.enter_context(tc.tile_pool(name="ovp", bufs=1, space="PSUM"))
            obp = ph2.enter_context(tc.tile_pool(name="obp", bufs=1, space="PSUM"))
            rbp = ph2.enter_context(tc.tile_pool(name="rbp", bufs=1, space="PSUM"))
            ep = ph2.enter_context(tc.tile_pool(name="ep", bufs=3))
            np_ = ph2.enter_context(tc.tile_pool(name="np", bufs=1))

            for qc in range(NSC):
                qs = slice(qc * SC, (qc + 1) * SC)
                oA0 = ovp.tile([128, SC], F32, tag="oA0")
                oA1 = ovp.tile([128, SC], F32, tag="oA1")
                # [0:32]=h0 tail dims, 32=h0 den, [64:96]=h1 tail, 96=h1 den
                oB = obp.tile([128, SC], F32, tag="oB")

                sc_tiles = {}

                def emit_scores(j):
                    js = slice(j * 128, (j + 1) * 128)
                    scps = scp.tile([128, 2 * SC], F32, tag="sc")
                    sc_tiles[j] = scps
                    nc.tensor.matmul(scps[:, 0:SC], KA0[:, js], QA0[:, qs],
                                     start=True, stop=False,
                                     skip_group_check=True)
                    nc.tensor.matmul(scps[:, SC:2 * SC], KA1[:, js], QA1[:, qs],
                                     start=True, stop=False,
                                     skip_group_check=True)
                    nc.tensor.matmul(scps[:, 0:SC], KB[0:32, js], QB[0:32, qs],
                                     start=False, stop=True,
                                     tile_position=(0, 0),
                                     skip_group_check=True)
                    nc.tensor.matmul(scps[:, SC:2 * SC], KB[32:64, js],
                                     QB[32:64, qs], start=False, stop=True,
                                     tile_position=(32, 0),
                                     skip_group_check=True)

                emit_scores(0)
                for j in range(NJ):
                    scps = sc_tiles.pop(j)
                    ex = ep.tile([128, 2 * SC], F16, tag="ex")
                    nc.scalar.activation(ex[:], scps[:], ExpF, bias=bias_t[:])
                    if j < NJ - 1:
                        emit_scores(j + 1)
                    st, sp_ = (j == 0), (j == NJ - 1)
                    nc.tensor.matmul(oA0[:], VA0[:, j, :], ex[:, 0:SC],
                                     start=st, stop=sp_, skip_group_check=True)
                    nc.tensor.matmul(oA1[:], VA1[:, j, :], ex[:, SC:2 * SC],
                                     start=st, stop=sp_, skip_group_check=True)
                    nc.tensor.matmul(oB[0:34, :], VB[:, j, 0:34], ex[:, 0:SC],
                                     start=st, stop=sp_, tile_position=(0, 0),
                                     skip_group_check=True)
                    nc.tensor.matmul(oB[64:98, :], VB[:, j, 34:68],
                                     ex[:, SC:2 * SC], start=st, stop=sp_,
                                     tile_position=(0, 64),
                                     skip_group_check=True)

                # normalize: rec = 1/den, broadcast via ones matmul, multiply
                rec0 = np_.tile([1, SC], F32, tag="rec0")
                rec1 = np_.tile([1, SC], F32, tag="rec1")
                nc.vector.reciprocal(rec0[:], oB[32:33, :])
                nc.vector.reciprocal(rec1[:], oB[96:97, :])
                rec0h = np_.tile([1, SC], F16, tag="rec0h")
                rec1h = np_.tile([1, SC], F16, tag="rec1h")
                nc.vector.tensor_copy(rec0h[:], rec0[:])
                nc.vector.tensor_copy(rec1h[:], rec1[:])
                rb0 = rbp.tile([128, SC], F32, tag="rb")
                nc.tensor.matmul(rb0[:], ones_rb[:], rec0h[:],
                                 start=True, stop=True)
                rbs0 = np_.tile([128, SC], F32, tag="rbs0")
                nc.vector.tensor_copy(rbs0[:], rb0[:])
                rb1 = rbp.tile([128, SC], F32, tag="rb")
                nc.tensor.matmul(rb1[:], ones_rb[:], rec1h[:],
                                 start=True, stop=True)
                rbs1 = np_.tile([128, SC], F32, tag="rbs1")
                nc.vector.tensor_copy(rbs1[:], rb1[:])
                nc.vector.tensor_mul(oTA0[:, qs], oA0[:], rbs0[:])
                nc.vector.tensor_mul(oTA1[:, qs], oA1[:], rbs1[:])
                nc.vector.tensor_mul(oTB[0:32, qs], oB[0:32, :], rbs0[0:32, :])
                nc.vector.tensor_mul(oTB[64:96, qs], oB[64:96, :],
                                     rbs1[64:96, :])

        # ---- phase 3: output projection (partial over this core's heads) ----
        with ExitStack() as ph3:
            fp = ph3.enter_context(tc.tile_pool(name="fp", bufs=2, space="PSUM"))
            fs = ph3.enter_context(tc.tile_pool(name="fs", bufs=2))
            for rq in range(S // 128):
                rs = slice(rq * 128, (rq + 1) * 128)
                osb = fs.tile([128, D], F32, tag="osb")
                for oc, osz in ((0, 512), (512, 512), (1024, 256)):
                    ps = fp.tile([128, osz], F32, tag=f"f{oc}")
                    nc.tensor.matmul(ps[:], oTA0[:, rs], WO[0][:, oc:oc + osz],
                                     start=True, stop=False)
                    nc.tensor.matmul(ps[:], oTA1[:, rs], WO[1][:, oc:oc + osz],
                                     start=False, stop=False)
                    nc.tensor.matmul(ps[:], oTB[:, rs], WO[2][:, oc:oc + osz],
                                     start=False, stop=True)
                    nc.any.tensor_copy(osb[:, oc:oc + osz], ps[:])
                nc.sync.dma_start(out[rs, :], osb[:])

    nc.compile()
    return nc


def kernel(hidden_states, w_q, w_k, w_v, lora_k_a, lora_k_b,
           lora_v_a, lora_v_b, w_out, b_out):
    f64 = np.float64
    wk_eff = (w_k.astype(f64)
              + w_k.astype(f64) @ lora_k_a.astype(f64) @ lora_k_b.astype(f64)
              ).astype(np.float32)
    wv_eff = (w_v.astype(f64)
              + w_v.astype(f64) @ lora_v_a.astype(f64) @ lora_v_b.astype(f64)
              ).astype(np.float32)
    wq_s = (w_q.astype(f64) / np.sqrt(DH)).astype(np.float32)
    w_out = np.asarray(w_out, np.float32)

    def pack_cols(w, h0, h1):
        # [h0 dims 0:128 | h1 dims 0:128 | h0 dims 128:160 | h1 dims 128:160]
        return np.concatenate([
            w[:, h0 * DH:h0 * DH + 128], w[:, h1 * DH:h1 * DH + 128],
            w[:, h0 * DH + 128:(h0 + 1) * DH],
            w[:, h1 * DH + 128:(h1 + 1) * DH]], axis=1)

    xT = [np.ascontiguousarray(np.asarray(hidden_states)[b].T
                               ).astype(np.float16) for b in range(B)]

    in_maps = []
    for c in range(N_CORES):
        b, p = c // 4, c % 4
        h0, h1 = 2 * p, 2 * p + 1
        wo_pack = np.zeros((384, D), np.float32)
        wo_pack[0:128] = w_out[h0 * DH:h0 * DH + 128]
        wo_pack[128:256] = w_out[h1 * DH:h1 * DH + 128]
        wo_pack[256:288] = w_out[h0 * DH + 128:(h0 + 1) * DH]
        wo_pack[320:352] = w_out[h1 * DH + 128:(h1 + 1) * DH]
        in_maps.append({
            "xT": xT[b],
            "wq": np.ascontiguousarray(pack_cols(wq_s, h0, h1)).astype(np.float16),
            "wk": np.ascontiguousarray(pack_cols(wk_eff, h0, h1)).astype(np.float16),
            "wv": np.ascontiguousarray(pack_cols(wv_eff, h0, h1)).astype(np.float16),
            "wo": wo_pack.astype(np.float16),
        })

    global _last_in_maps
    _last_in_maps = in_maps
    if "nc" not in _CACHE:
        _CACHE["nc"] = build()
    res = run_bass_kernel_spmd(_CACHE["nc"], in_maps, list(range(N_CORES)))

    out = np.zeros((B, S, D), np.float32)
    for c in range(N_CORES):
        out[c // 4] += res.results[c]["out"]
    out += np.asarray(b_out, np.float32)
    return out
